# revision 1
# baseline (speedup 1.0000x reference)
"""Weighted Chamfer loss on Trainium2 (8 NeuronCores, batch-parallel).

Problem (per batch element b of 8):
    dist[i, j] = || set1[b, i] - set2[b, j] ||_2            (4096 x 4096, C=128)
    total = (sum_i w1[b,i] * min_j dist + sum_j w2[b,j] * min_i dist) / 2

Sharding: one batch element per NeuronCore (pure data parallel, no
collectives); the 8 per-core partial sums are added on the host.

Per-core pipeline, per [128 x 2048] PSUM unit (x row-block b, y col-half h):
  PE    : psum = x@y^T - x2/2 - y2/2 via fp16 matmuls (fp32 PSUM accum):
          4 main matmuls plus 4 rank-2 "bake" matmuls whose operands are
          zero-padded to K=128 (small-K weight loads are slow on HW);
          bake lhsT rows are [-x2/2 ; 1 ; 0...], rhs rows [1 ; -y2/2 ; 0...].
  ACT   : evacuates PSUM to SBUF fp16 with Identity(scale=-2) -> full d^2.
  DVE   : fp16 tensor_tensor(min) folds the block into the column-min
          accumulator (2x mode); a 2-level pairwise min fold + one strided
          tensor_reduce(min) produce the block row-min.
  Tail  : PE transposes of the column-min accumulator + min reduce give
          per-column mins; ACT relu+sqrt; DVE mul + sum-reduce for the
          weighted partials, summed on host.
"""

import sys
from contextlib import ExitStack, nullcontext

import numpy as np

for _p in ("/opt/trn_rl_repo",):
    if _p not in sys.path:
        sys.path.insert(0, _p)

import concourse.bass as bass
import concourse.tile as tile
from concourse import bacc, masks, mybir
from concourse.bass_utils import run_bass_kernel_spmd

AF = mybir.ActivationFunctionType
ALU = mybir.AluOpType
DT = mybir.dt

N_CORES = 8
N = 4096          # points per set per batch element
C = 128           # channels (= contraction dim = partition dim)
NB = N // 128     # 32 row blocks of x
UCOLS = 2048      # y columns per PSUM unit (half of PSUM)
NH = N // UCOLS   # 2 column halves
MMN = 512         # moving free dim per matmul (one fp32 PSUM bank)
NT = UCOLS // 128 # 16 transpose tiles per column half

_CACHE = {}
LAST_RESULTS = None  # BassKernelResults of the most recent run (for profiling)

def _build_program(repeat=1, parts="pe,act,dve"):
    nc = bacc.Bacc(
        "TRN2", debug=False, target_bir_lowering=False, num_devices=N_CORES
    )
    xt_d = nc.dram_tensor("xt", [C, N], DT.float32, kind="ExternalInput").ap()
    yt_d = nc.dram_tensor("yt", [C, N], DT.float32, kind="ExternalInput").ap()
    on_d = nc.dram_tensor("ones_row", [1, N], DT.float16, kind="ExternalInput").ap()
    w1t_d = nc.dram_tensor("w1t", [128, NB], DT.float32, kind="ExternalInput").ap()
    w2t_d = nc.dram_tensor("w2t", [128, NB], DT.float32, kind="ExternalInput").ap()
    out_d = nc.dram_tensor("out", [128, 2], DT.float32, kind="ExternalOutput").ap()

    en_act = "act" in parts
    en_dve = "dve" in parts

    with tile.TileContext(nc) as tc, ExitStack() as ctx:
        persist = ctx.enter_context(tc.tile_pool(name="persist", bufs=1))
        prep = ctx.enter_context(tc.tile_pool(name="prep", bufs=2))
        d2p = ctx.enter_context(tc.tile_pool(name="d2", bufs=8))
        psum = ctx.enter_context(tc.tile_pool(name="psum", bufs=2, space="PSUM"))

        # ---------------- inputs + fp16 casts ----------------
        xt = persist.tile([C, N], DT.float32)
        yt = persist.tile([C, N], DT.float32)
        nc.sync.dma_start(xt[:], xt_d[:])
        nc.sync.dma_start(yt[:], yt_d[:])
        w1t = persist.tile([128, NB], DT.float32)
        w2t = persist.tile([128, NB], DT.float32)
        nc.sync.dma_start(w1t[:], w1t_d[:])
        nc.sync.dma_start(w2t[:], w2t_d[:])

        xth = persist.tile([C, N], DT.float16)
        yth = persist.tile([C, N], DT.float16)
        nc.vector.tensor_copy(xth[:], xt[:])
        nc.vector.tensor_copy(yth[:], yt[:])

        identity = persist.tile([128, 128], DT.float16)
        masks.make_identity(nc, identity[:])

        ones = persist.tile([C, 1], DT.float16)
        nc.gpsimd.memset(ones[:], 1.0)

        colacc = persist.tile([128, N], DT.float16)
        nc.gpsimd.memset(colacc[:], 60000.0)

        rm = [
            persist.tile([128, NB], DT.float32, name=f"rm{i}", tag=f"rm{i}")
            for i in range(NH)
        ]
        colminT = persist.tile([128, NB], DT.float32)

        # bake operands, K padded to 128 with zero rows (small-K weight
        # loads measure ~500ns extra per matmul on HW):
        # lhsT rows [-x2/2 ; 1 ; 0...], rhs rows [1 ; -y2/2 ; 0...]
        bake_lhs = persist.tile([C, N], DT.float16)
        bake_rhs = persist.tile([C, N], DT.float16)
        nc.vector.memset(bake_lhs[:], 0.0)
        nc.vector.memset(bake_rhs[:], 0.0)
        nc.sync.dma_start(bake_lhs[1:2, :], on_d[:])
        nc.sync.dma_start(bake_rhs[0:1, :], on_d[:])

        # ---------------- squared norms ----------------
        # sq = t*t (ACT Square -> fp16), column-reduce over channels via
        # ones-matmul into PSUM row 0, then Identity(scale=-0.5) -> fp16 row;
        # x2 lands directly in bake_lhs row 0, y2 bounces via an SBUF->SBUF
        # DMA into bake_rhs row 1 (engines cannot write partition 1).
        norm_y = persist.tile([1, N], DT.float16)
        for src, dst, row in ((xt, bake_lhs, 0), (yt, norm_y, 1)):
            sq = prep.tile([C, N], DT.float16, tag="sq", name=f"sq{row}")
            nc.scalar.activation(sq[:], src[:], AF.Square)
            for half in range(NH):
                ps = psum.tile(
                    [128, UCOLS], DT.float32, tag="unit", name=f"nps{row}{half}"
                )
                for k in range(UCOLS // MMN):
                    c0 = k * MMN
                    nc.tensor.matmul(
                        ps[0:1, c0 : c0 + MMN],
                        ones[:],
                        sq[:, half * UCOLS + c0 : half * UCOLS + c0 + MMN],
                        start=True,
                        stop=True,
                    )
                nc.scalar.activation(
                    dst[0:1, half * UCOLS : (half + 1) * UCOLS],
                    ps[0:1, :],
                    AF.Identity,
                    scale=-0.5,
                )
        nc.sync.dma_start(bake_rhs[1:2, :], norm_y[:])

        if not en_dve:
            for t in rm:
                nc.gpsimd.memset(t[:], 1.0)

        with tc.For_i(0, repeat, 1) if repeat > 1 else nullcontext():
            # ---------------- main loop ----------------
            for h in range(NH):
                ycols = slice(h * UCOLS, (h + 1) * UCOLS)
                for b in range(NB):
                    ps = psum.tile([128, UCOLS], DT.float32, tag="unit")
                    for k in range(UCOLS // MMN):
                        c0 = k * MMN
                        nc.tensor.matmul(
                            ps[:, c0 : c0 + MMN],
                            xth[:, b * 128 : (b + 1) * 128],
                            yth[:, h * UCOLS + c0 : h * UCOLS + c0 + MMN],
                            start=True,
                            stop=False,
                        )
                    for k in range(UCOLS // MMN):
                        c0 = k * MMN
                        nc.tensor.matmul(
                            ps[:, c0 : c0 + MMN],
                            bake_lhs[:, b * 128 : (b + 1) * 128],
                            bake_rhs[:, h * UCOLS + c0 : h * UCOLS + c0 + MMN],
                            start=False,
                            stop=True,
                        )
                    d2 = d2p.tile([128, UCOLS], DT.float16, tag="d2")
                    if en_act:
                        if "esplit" in parts:
                            hw_ = UCOLS // 2
                            nc.scalar.activation(
                                d2[:, 0:hw_], ps[:, 0:hw_], AF.Identity, scale=-2.0
                            )
                            nc.scalar.activation(
                                d2[:, hw_:], ps[:, hw_:], AF.Identity, scale=-2.0
                            )
                        else:
                            nc.scalar.activation(
                                d2[:], ps[:], AF.Identity, scale=-2.0
                            )
                    if en_dve:
                        nc.vector.tensor_tensor(
                            colacc[:, ycols], d2[:], colacc[:, ycols], ALU.min
                        )
                        # pairwise fold, then one strided min-reduce
                        nfold = 2
                        for p in parts.split(","):
                            if p.startswith("fold"):
                                nfold = int(p[4:])
                        w = UCOLS // 2
                        for _ in range(nfold):
                            nc.vector.tensor_tensor(
                                d2[:, 0:w], d2[:, 0:w], d2[:, w : 2 * w], ALU.min
                            )
                            w //= 2
                        w *= 2
                        nc.vector.tensor_reduce(
                            rm[h][:, b : b + 1],
                            d2[:, 0:w].rearrange("p (t c) -> p t c", c=min(w, 512)),
                            axis=mybir.AxisListType.XY,
                            op=ALU.min,
                        )

            # column mins: PE-transpose 128-blocks of colacc into PSUM
            # units, then one strided min-reduce per half. Emitted after the
            # full loop so they never steal a main-unit PSUM slot mid-loop.
            for h in range(NH):
                pst = psum.tile([128, UCOLS], DT.float16, tag="unit", name="pst")
                for t in range(NT):
                    nc.tensor.transpose(
                        pst[:, t * 128 : (t + 1) * 128],
                        colacc[:, h * UCOLS + t * 128 : h * UCOLS + (t + 1) * 128],
                        identity[:],
                    )
                nc.vector.tensor_reduce(
                    colminT[:, h * NT : (h + 1) * NT],
                    pst[:].rearrange("p (t c) -> p t c", c=128),
                    axis=mybir.AxisListType.X,
                    op=ALU.min,
                )

            # ---------------- tail ----------------
            rowmin = persist.tile([128, NB], DT.float32)
            nc.vector.tensor_tensor(rowmin[:], rm[0][:], rm[1][:], ALU.min)

            rowr = persist.tile([128, NB], DT.float32)
            rowd = persist.tile([128, NB], DT.float32)
            nc.scalar.activation(rowr[:], rowmin[:], AF.Relu)
            nc.scalar.activation(rowd[:], rowr[:], AF.Sqrt)

            colr = persist.tile([128, NB], DT.float32)
            cold = persist.tile([128, NB], DT.float32)
            nc.scalar.activation(colr[:], colminT[:], AF.Relu)
            nc.scalar.activation(cold[:], colr[:], AF.Sqrt)

            junk = persist.tile([128, NB], DT.float32)
            outacc = persist.tile([128, 2], DT.float32)
            nc.vector.tensor_mul(junk[:], rowd[:], w1t[:])
            nc.vector.tensor_reduce(
                outacc[:, 0:1], junk[:], axis=mybir.AxisListType.X, op=ALU.add
            )
            junk2 = persist.tile([128, NB], DT.float32)
            nc.vector.tensor_mul(junk2[:], cold[:], w2t[:])
            nc.vector.tensor_reduce(
                outacc[:, 1:2], junk2[:], axis=mybir.AxisListType.X, op=ALU.add
            )
            nc.sync.dma_start(out_d[:], outacc[:])

    nc.compile()
    return nc


def _get_nc(repeat=1, parts="pe,act,dve"):
    key = ("nc", repeat, parts)
    if key not in _CACHE:
        _CACHE[key] = _build_program(repeat, parts)
    return _CACHE[key]


def _make_in_maps(set1, set2, w1, w2):
    ones_row = np.ones((1, N), dtype=np.float16)
    in_maps = []
    for b in range(N_CORES):
        in_maps.append(
            {
                "xt": np.ascontiguousarray(set1[b].T, dtype=np.float32),
                "yt": np.ascontiguousarray(set2[b].T, dtype=np.float32),
                "ones_row": ones_row,
                "w1t": np.ascontiguousarray(
                    w1[b].reshape(NB, 128).T, dtype=np.float32
                ),
                "w2t": np.ascontiguousarray(
                    w2[b].reshape(NB, 128).T, dtype=np.float32
                ),
            }
        )
    return in_maps


def kernel(set1, set2, w1, w2):
    global LAST_RESULTS
    set1 = np.asarray(set1, dtype=np.float32)
    set2 = np.asarray(set2, dtype=np.float32)
    w1 = np.asarray(w1, dtype=np.float32)
    w2 = np.asarray(w2, dtype=np.float32)

    nc = _get_nc()
    in_maps = _make_in_maps(set1, set2, w1, w2)
    res = run_bass_kernel_spmd(nc, in_maps, core_ids=list(range(N_CORES)))
    LAST_RESULTS = res

    total = 0.0
    for core_out in res.results:
        total += float(core_out["out"].astype(np.float64).sum())
    return np.float32(total / 2.0)

